# revision 20
# baseline (speedup 1.0000x reference)
"""Trainium2 Bass kernel for nn_ContrastiveLoss (patch-level contrastive loss).

Reference math:
  n1 = normalize(normal_embed)  [N,P,D], n2 = normalize(defect_embed) [M,P,D]
  sim_nn[i,j,q] = max_p <n1[i,p,:], n1[j,q,:]>   (max over first arg's patches)
  sim_nd[i,j,q] = max_p <n1[i,p,:], n2[j,q,:]>
  pos_loss = sum_{i<j,q} (1 - sim_nn[i,j,q]) / (npairs*P)
  neg_loss = mean(relu(sim_nd - 0.5))
  loss = pos_loss + neg_loss

Distribution (8 NeuronCores, data-parallel over i):
  Core c owns moving pairs A=(2c, 2c+1), B=(30-2c, 31-2c). Embeddings are
  normalized on host, scaled by S, quantized to fp8e4m3 and shipped as
  uint8 (bitcast to float8e4 at the matmul). Matmuls run in DoubleRow perf
  mode: each instruction contracts TWO 128-deep k-chunks ([128,2,*] APs),
  so D=768 takes 3 matmuls per 392-wide PSUM bank.

  The j-side streams as 128-wide stationary q-tiles against 392-wide moving
  halves (one image pair per bank, two banks per unit). The nn side
  exploits the i<j triangle with a core-uniform schedule (single SPMD
  program):
    - flex: 24 host-packed per-core q-tiles vs BOTH pairs (exactly the
      per-core leftovers: (25-tA)+(47-tB) == 24 for every core); the wmask
      kills the half that doesn't apply. Runs FIRST: flex consumes one
      stationary slot per unit, letting the DMA stream get ahead.
    - fixed-A: q-tiles 25..48 vs pair A (every core's pair A needs them all
      since max_c tileof(2c+1) = 22 < 25),
    - fixed-B: q-tiles 47,48 vs pair B,
  plus the full 49-tile sweep for the nd side.

  Max-over-p runs straight out of PSUM on a rotating mix of engines
  (pattern-tunable): B = GpSimd tensor_max halves PSUM->SBUF f32 and DVE
  reduce_max finishes; C = Act copies PSUM->SBUF bf16 and DVE tensor_max +
  reduce_max finish; D = Act copies, GpSimd halves, DVE finishes; V = DVE
  reduce_max direct. Warmup matmuls on zeros ramp the PE p-state during the
  initial DMA fill. Finals are segmented (fused multiply/relu-accumulate)
  so only the last segment sits in the tail; a ones-matmul folds partitions
  and the host combines per-core partials.
"""

import os

import numpy as np

# Problem constants (hardcoded per the contract; kernel.py is self-contained).
N_IMG = 32
P = 196
D = 768
EPS = 1e-8
MARGIN = 0.5
NCORES = 8
NT = N_IMG * P // 128       # 49 stationary q-tiles per side (exact: 6272/128)
NPAIRS = N_IMG * (N_IMG - 1) // 2

# fp8 scale: sims come out multiplied by SCALE^2; undone on host.
SCALE = 16.0

# Triangular-nn schedule (1) vs full NxN (0).
TRI = os.environ.get("CL_TRI", "1") == "1"
# Reduce pipeline mode per unit, cyclic pattern (see module docstring).
RED_PATTERN = os.environ.get("CL_RED", "BCBBDBCBBCBBD")
# PE p-state warmup matmuls issued before the first real unit.
WARM = int(os.environ.get("CL_WARM", "16"))
# PSUM pool depth (tiles of 2 banks each; 4 uses all 8 banks).
PSUM_BUFS = int(os.environ.get("CL_PSUM", "4"))

# Per-core pair bases: pair A = (2c, 2c+1), pair B = (30-2c, 31-2c).
TA = [((2 * c + 1) * P) // 128 for c in range(NCORES)]   # first tile pair A needs
TB = [((31 - 2 * c) * P) // 128 for c in range(NCORES)]  # first tile pair B needs
FIXED_A_T0 = 23   # fixed-A section: tiles 23..48 (>= max(TA)=22)
FIXED_B_T0 = 47   # fixed-B section: tiles 47,48 (>= max(TB)=47)
N_FIXED_A = NT - FIXED_A_T0          # 26
N_FLEX = 22
for _c in range(NCORES):
    assert (FIXED_A_T0 - TA[_c]) + (FIXED_B_T0 - TB[_c]) == N_FLEX
NN_SLOTS = N_FLEX + N_FIXED_A        # 48 packed stationary nn tiles
# fixed-A units that run before flex (their slots lead the packed layout so
# compute can start as soon as mov pair A + the first slots arrive).
N_FA_HEAD = 3

_CACHE = {}


def _pairs(c):
    return (2 * c, 2 * c + 1), (30 - 2 * c, 31 - 2 * c)


def _flex_tiles(c):
    """Per-core flex q-tiles: pair-A leftovers then pair-B leftovers."""
    return list(range(TA[c], FIXED_A_T0)) + list(range(TB[c], FIXED_B_T0))


def _slot2tile(c):
    """Packed nn stationary layout: head fixed-A tiles, then per-core flex
    tiles, then the remaining fixed-A tiles (47,48 double as fixed-B)."""
    head = list(range(FIXED_A_T0, FIXED_A_T0 + 2 * N_FA_HEAD))
    rest = list(range(FIXED_A_T0 + 2 * N_FA_HEAD, NT))
    return head + _flex_tiles(c) + rest


def _schedule():
    """Core-independent unit list. Each unit: side 0 nn / 1 nd; banks: list
    of (slot, half); half 0 = moving cols 0:392 (pair A), 1 = 392:784 (pair
    B). Slot indexes 128-col groups of that side's packed stationary."""
    units = []
    if TRI:
        # fixed-A head: starts as soon as mov pair A + first slots land
        for u in range(N_FA_HEAD):
            units.append({"side": 0, "banks": [(2 * u, 0), (2 * u + 1, 0)]})
        # flex: one slot per unit (lets the DMA stream get ahead)
        for k in range(N_FLEX):
            s = 2 * N_FA_HEAD + k
            units.append({"side": 0, "banks": [(s, 0), (s, 1)]})
        # fixed-A rest
        for u in range((N_FIXED_A - 2 * N_FA_HEAD) // 2):
            s = 2 * N_FA_HEAD + N_FLEX + 2 * u
            units.append({"side": 0, "banks": [(s, 0), (s + 1, 0)]})
        # fixed-B: tiles 47,48 = last two slots, pair B
        units.append({"side": 0, "banks": [(NN_SLOTS - 2, 1), (NN_SLOTS - 1, 1)]})
    else:
        for t in range(NT):
            units.append({"side": 0, "banks": [(t, 0), (t, 1)]})
    for t in range(NT):
        units.append({"side": 1, "banks": [(t, 0), (t, 1)]})
    return units


SCHED = _schedule()
NN_UNITS = sum(1 for u in SCHED if u["side"] == 0)
NN_COLS = 4 * NN_UNITS
NCOLS = 4 * len(SCHED)
NN_STAT_COLS = (NN_SLOTS if TRI else NT) * 128
ND_STAT_COLS = NT * 128
# final-sum segments: (first_unit, last_unit_exclusive, is_pos). The last
# segment is small so only its drain sits in the serial tail.
ND0 = NN_UNITS
_NDTAIL = max(ND0, len(SCHED) - 6)
SEGMENTS = [
    (0, NN_UNITS, True),
    (ND0, _NDTAIL, False),
    (_NDTAIL, len(SCHED), False),
]


def _red_mode_seq(n):
    seq = [RED_PATTERN[u % len(RED_PATTERN)] for u in range(n)]
    # Last units drain on three different engines in parallel so the tail
    # isn't a single-engine backlog.
    seq[-3:] = ["C", "B", "V"]
    return seq


def _build_nc():
    import concourse.bacc as bacc
    import concourse.mybir as mybir
    import concourse.tile as tile

    f32 = mybir.dt.float32
    bf16 = mybir.dt.bfloat16
    u8 = mybir.dt.uint8
    f8 = mybir.dt.float8e4
    DR = mybir.MatmulPerfMode.DoubleRow
    HP = P // 2  # 98: max-halving split

    # Bacc (not plain Bass): its compile() runs move_matmul_waits_to_ldweights
    # + generate_event_semaphores, which legalize multi-semaphore waits for
    # the 1-wait-per-instruction ISA constraint.
    nc = bacc.Bacc("TRN2", target_bir_lowering=False, debug=False)

    mov_d = nc.dram_tensor("mov", [D, 4 * P], u8, kind="ExternalInput")
    stat_nn_d = nc.dram_tensor("stat_nn", [D, NN_STAT_COLS], u8, kind="ExternalInput")
    stat_nd_d = nc.dram_tensor("stat_nd", [D, ND_STAT_COLS], u8, kind="ExternalInput")
    wmask_d = nc.dram_tensor("wmask", [128, NN_COLS], f32, kind="ExternalInput")
    out_d = nc.dram_tensor("out", [1, len(SEGMENTS)], f32, kind="ExternalOutput")

    red_mode = _red_mode_seq(len(SCHED))

    with tile.TileContext(nc) as tc:
        with (
            tc.tile_pool(name="const", bufs=1) as const_pool,
            tc.tile_pool(name="statp", bufs=1) as stat_pool,
            tc.tile_pool(name="slots", bufs=1) as slot_pool,
            tc.tile_pool(name="stageB", bufs=3) as stageB_pool,
            tc.tile_pool(name="stageC", bufs=3) as stageC_pool,
            tc.tile_pool(name="psum", bufs=PSUM_BUFS, space="PSUM") as psum_pool,
        ):
            # Moving pair A first (the head units need only it), then the
            # first stationary slots, then pair B, then the rest.
            mov_sb = const_pool.tile([128, 6, 4 * P], u8)
            stat_nn_sb = stat_pool.tile([128, 6, NN_STAT_COLS], u8)
            stat_nd_sb = stat_pool.tile([128, 6, ND_STAT_COLS], u8)

            def _mov_chunk(lo, hi):
                nc.sync.dma_start(
                    mov_sb[:, :, lo:hi],
                    mov_d[:, lo:hi].rearrange("(c k) p -> k c p", k=128),
                )

            def _stat_chunks(dram, sbuf, bounds):
                for lo, hi in zip(bounds[:-1], bounds[1:]):
                    src = dram[:, 128 * lo : 128 * hi].rearrange(
                        "(c k) q -> k c q", k=128
                    )
                    nc.sync.dma_start(sbuf[:, :, 128 * lo : 128 * hi], src)

            _mov_chunk(0, 392)
            _stat_chunks(stat_nn_d, stat_nn_sb, [0, 2, 6])
            _mov_chunk(392, 784)
            _stat_chunks(stat_nn_d, stat_nn_sb,
                         [6, 16, 28, 38, NN_STAT_COLS // 128])
            _stat_chunks(stat_nd_d, stat_nd_sb, [0, 12, 25, 37, NT])
            stat_sbs = (stat_nn_sb, stat_nd_sb)

            wmask_sb = const_pool.tile([128, NN_COLS], f32)
            nc.sync.dma_start(wmask_sb[:], wmask_d[:, :])

            ones_sb = const_pool.tile([128, 1], f32)
            nc.vector.memset(ones_sb[:], 1.0)
            mslots = slot_pool.tile([128, NCOLS], f32, name="mslots")
            acc = const_pool.tile([128, len(SEGMENTS)], f32)

            # PE p-state warmup on zeros while the first DMAs land.
            if WARM:
                zeros_sb = const_pool.tile([128, 2, 392], u8)
                nc.gpsimd.memset(zeros_sb[:], 0)
                pw = psum_pool.tile([128, 2, 512], f32, tag="ps")
                for _ in range(WARM):
                    nc.tensor.matmul(
                        pw[:, 0, 0:392],
                        zeros_sb[:, :, 0:128].bitcast(f8),
                        zeros_sb[:].bitcast(f8),
                        start=True,
                        stop=True,
                        perf_mode=DR,
                    )

            seg_of_unit = {}
            for si, (u0, u1, _) in enumerate(SEGMENTS):
                for u in range(u0, u1):
                    seg_of_unit[u] = si
            max_seg = max(4 * (u1 - u0) for u0, u1, _ in SEGMENTS)
            junk = slot_pool.tile([128, max_seg], f32, name="junk")

            for u, unit in enumerate(SCHED):
                ps = psum_pool.tile([128, 2, 512], f32, tag="ps")
                for b, (slot, half) in enumerate(unit["banks"]):
                    stat_sb = stat_sbs[unit["side"]]
                    for t3 in range(3):
                        lhsT = stat_sb[
                            :, 2 * t3 : 2 * t3 + 2, 128 * slot : 128 * (slot + 1)
                        ].bitcast(f8)
                        rhs = mov_sb[
                            :, 2 * t3 : 2 * t3 + 2, 392 * half : 392 * half + 392
                        ].bitcast(f8)
                        nc.tensor.matmul(
                            ps[:, b, 0:392],
                            lhsT,
                            rhs,
                            start=(t3 == 0),
                            stop=(t3 == 2),
                            perf_mode=DR,
                        )
                mview = ps[:, :, 0:392].rearrange("k b (i p) -> k b i p", p=P)
                mout = mslots[:, 4 * u : 4 * u + 4]
                mode = red_mode[u]
                if mode == "V":
                    nc.vector.reduce_max(
                        out=mout, in_=mview, axis=mybir.AxisListType.X
                    )
                elif mode == "B":
                    h = stageB_pool.tile([128, 2, 2, HP], f32, tag="hB")
                    nc.gpsimd.tensor_max(
                        h[:], mview[:, :, :, 0:HP], mview[:, :, :, HP:P]
                    )
                    nc.vector.reduce_max(
                        out=mout, in_=h[:], axis=mybir.AxisListType.X
                    )
                else:  # "C" / "D"
                    hc = stageC_pool.tile([128, 2, 2, P], bf16, tag="hC")
                    nc.scalar.copy(hc[:], mview)
                    h2 = stageC_pool.tile([128, 2, 2, HP], bf16, tag="hC2")
                    eng = nc.vector if mode == "C" else nc.gpsimd
                    eng.tensor_max(
                        h2[:], hc[:, :, :, 0:HP], hc[:, :, :, HP:P]
                    )
                    h3 = stageC_pool.tile([128, 2, 2, HP // 2], bf16, tag="hC3")
                    nc.vector.tensor_max(
                        h3[:], h2[:, :, :, 0 : HP // 2], h2[:, :, :, HP // 2 : HP]
                    )
                    nc.vector.reduce_max(
                        out=mout, in_=h3[:], axis=mybir.AxisListType.X
                    )

                # segment finals, fused accumulate, off the tail
                for si, (u0, u1, is_pos) in enumerate(SEGMENTS):
                    if u != u1 - 1:
                        continue
                    c0, c1 = 4 * u0, 4 * u1
                    if is_pos:
                        nc.vector.scalar_tensor_tensor(
                            out=junk[:, 0 : c1 - c0],
                            in0=mslots[:, c0:c1],
                            scalar=1.0,
                            in1=wmask_sb[:, c0:c1],
                            op0=mybir.AluOpType.mult,
                            op1=mybir.AluOpType.mult,
                            accum_out=acc[:, si : si + 1],
                        )
                    else:
                        nc.vector.tensor_scalar(
                            out=junk[:, 0 : c1 - c0],
                            in0=mslots[:, c0:c1],
                            scalar1=-MARGIN * SCALE * SCALE,
                            scalar2=0.0,
                            op0=mybir.AluOpType.add,
                            op1=mybir.AluOpType.max,
                            accum_out=acc[:, si : si + 1],
                        )

            # partition reduction via ones-matmul into a rotated psum tile
            ps_f = psum_pool.tile([128, 2, 512], f32, tag="ps")
            nc.tensor.matmul(
                ps_f[0:1, 0, 0 : len(SEGMENTS)],
                ones_sb[:],
                acc[:],
                start=True,
                stop=True,
            )
            out_sb = const_pool.tile([1, len(SEGMENTS)], f32)
            nc.vector.tensor_copy(out_sb[:], ps_f[0:1, 0, 0 : len(SEGMENTS)])
            nc.sync.dma_start(out_d[:, :], out_sb[:])

    nc.compile()
    return nc


def _quant(n):
    """[*, D] fp32 normalized -> fp8e4m3 bytes of n*SCALE."""
    import ml_dtypes

    return (n * SCALE).astype(ml_dtypes.float8_e4m3).view(np.uint8)


def _statq(n):
    """[32,P,D] -> d-major [D, 32*P] (j-major q axis)."""
    return np.ascontiguousarray(n.transpose(2, 0, 1).reshape(D, N_IMG * P))


def _build_in_maps(normal_embed, defect_embed):
    x1 = np.asarray(normal_embed, dtype=np.float32)
    x2 = np.asarray(defect_embed, dtype=np.float32)
    n1 = x1 / (np.sqrt(np.sum(x1 * x1, axis=-1, keepdims=True)) + EPS)
    n2 = x2 / (np.sqrt(np.sum(x2 * x2, axis=-1, keepdims=True)) + EPS)

    q1 = _statq(_quant(n1))  # [D, 6272] uint8 view of fp8
    q2 = _statq(_quant(n2))

    jq = np.arange(NT * 128) // P  # j image per stationary q

    in_maps = []
    for c in range(NCORES):
        pa, pb = _pairs(c)
        imgs = [pa[0], pa[1], pb[0], pb[1]]
        mov = np.ascontiguousarray(
            np.concatenate([q1[:, i * P : (i + 1) * P] for i in imgs], axis=1)
        )

        if TRI:
            slot2tile = _slot2tile(c)
            stat_nn = np.ascontiguousarray(
                np.concatenate(
                    [q1[:, 128 * t : 128 * (t + 1)] for t in slot2tile], axis=1
                )
            )
        else:
            stat_nn = q1
            slot2tile = list(range(NT))

        wm = np.zeros((128, NN_COLS), dtype=np.float32)
        for u, unit in enumerate(SCHED):
            if unit["side"] != 0:
                continue
            for b, (slot, half) in enumerate(unit["banks"]):
                t = slot2tile[slot]
                pair = pa if half == 0 else pb
                # flex double-count guard: the fixed sections are the unique
                # cover for tiles >= FIXED_A_T0 (pair A) / >= FIXED_B_T0
                # (pair B), so flex contributes pair A only below FIXED_A_T0
                # and pair B only at/above it (flex-B tiles are < FIXED_B_T0).
                if TRI and N_FA_HEAD <= u < N_FA_HEAD + N_FLEX:
                    ok = (t < FIXED_A_T0) if half == 0 else (t >= FIXED_A_T0)
                    if not ok:
                        continue
                q = 128 * t + np.arange(128)
                for m in range(2):
                    col = 4 * u + 2 * b + m
                    wm[:, col] = (jq[q] > pair[m]).astype(np.float32)
        assert int(wm.sum()) == 62 * P, (c, int(wm.sum()))

        in_maps.append(
            {
                "mov": mov,
                "stat_nn": stat_nn,
                "stat_nd": q2,
                "wmask": np.ascontiguousarray(wm),
            }
        )
    return in_maps


def _get_nc():
    key = ("nc", TRI, RED_PATTERN, WARM, PSUM_BUFS)
    if key not in _CACHE:
        _CACHE[key] = _build_nc()
    return _CACHE[key]


def _run_on_device(in_maps, trace=False):
    from concourse.bass_utils import run_bass_kernel_spmd

    nc = _get_nc()
    return run_bass_kernel_spmd(
        nc, in_maps, core_ids=list(range(NCORES)), trace=trace
    )


def _combine(results):
    s_pos = 0.0
    s_neg = 0.0
    for r in results:
        o = np.asarray(r["out"], dtype=np.float64).reshape(-1)
        for si, (_, _, is_pos) in enumerate(SEGMENTS):
            if is_pos:
                s_pos += float(o[si])
            else:
                s_neg += float(o[si])
    s2 = SCALE * SCALE
    loss = 1.0 - s_pos / s2 / (NPAIRS * P) + s_neg / s2 / (N_IMG * N_IMG * P)
    return np.float32(loss)


def kernel(normal_embed, defect_embed):
    in_maps = _build_in_maps(normal_embed, defect_embed)
    res = _run_on_device(in_maps, trace=False)
    return _combine(res.results)


# revision 22
# speedup vs baseline: 1.1202x; 1.1202x over previous
"""Trainium2 Bass kernel for nn_ContrastiveLoss (patch-level contrastive loss).

Reference math:
  n1 = normalize(normal_embed)  [N,P,D], n2 = normalize(defect_embed) [M,P,D]
  sim_nn[i,j,q] = max_p <n1[i,p,:], n1[j,q,:]>   (max over first arg's patches)
  sim_nd[i,j,q] = max_p <n1[i,p,:], n2[j,q,:]>
  pos_loss = sum_{i<j,q} (1 - sim_nn[i,j,q]) / (npairs*P)
  neg_loss = mean(relu(sim_nd - 0.5))
  loss = pos_loss + neg_loss

Distribution (8 NeuronCores, data-parallel over i):
  Core c owns moving pairs A=(2c, 2c+1), B=(30-2c, 31-2c). Embeddings are
  normalized on host, scaled by S, quantized to fp8e4m3 and shipped as
  uint8 (bitcast to float8e4 at the matmul). Matmuls run in DoubleRow perf
  mode: each instruction contracts TWO 128-deep k-chunks ([128,2,*] APs),
  so D=768 takes 3 matmuls per 392-wide PSUM bank.

  The j-side streams as 128-wide stationary q-tiles against 392-wide moving
  halves (one image pair per bank, two banks per unit). The nn side
  exploits the i<j triangle with a core-uniform schedule (single SPMD
  program):
    - flex: 24 host-packed per-core q-tiles vs BOTH pairs (exactly the
      per-core leftovers: (25-tA)+(47-tB) == 24 for every core); the wmask
      kills the half that doesn't apply. Runs FIRST: flex consumes one
      stationary slot per unit, letting the DMA stream get ahead.
    - fixed-A: q-tiles 25..48 vs pair A (every core's pair A needs them all
      since max_c tileof(2c+1) = 22 < 25),
    - fixed-B: q-tiles 47,48 vs pair B,
  plus the full 49-tile sweep for the nd side.

  Max-over-p runs straight out of PSUM on a rotating mix of engines
  (pattern-tunable): B = GpSimd tensor_max halves PSUM->SBUF f32 and DVE
  reduce_max finishes; C = Act copies PSUM->SBUF bf16 and DVE tensor_max +
  reduce_max finish; D = Act copies, GpSimd halves, DVE finishes; V = DVE
  reduce_max direct. Warmup matmuls on zeros ramp the PE p-state during the
  initial DMA fill. Finals are segmented (fused multiply/relu-accumulate)
  so only the last segment sits in the tail; a ones-matmul folds partitions
  and the host combines per-core partials.
"""

import os

import numpy as np

# Problem constants (hardcoded per the contract; kernel.py is self-contained).
N_IMG = 32
P = 196
D = 768
EPS = 1e-8
MARGIN = 0.5
NCORES = 8
NT = N_IMG * P // 128       # 49 stationary q-tiles per side (exact: 6272/128)
NPAIRS = N_IMG * (N_IMG - 1) // 2

# fp8 scale: sims come out multiplied by SCALE^2; undone on host.
SCALE = 16.0

# Triangular-nn schedule (1) vs full NxN (0).
TRI = os.environ.get("CL_TRI", "1") == "1"
# Reduce pipeline mode per unit, cyclic pattern (see module docstring).
RED_PATTERN = os.environ.get("CL_RED", "BCBBDBCBBCBBD")
# PE p-state warmup matmuls issued before the first real unit.
WARM = int(os.environ.get("CL_WARM", "16"))
# PSUM pool depth (tiles of 2 banks each; 4 uses all 8 banks).
PSUM_BUFS = int(os.environ.get("CL_PSUM", "4"))

# Per-core pair bases: pair A = (2c, 2c+1), pair B = (30-2c, 31-2c).
TA = [((2 * c + 1) * P) // 128 for c in range(NCORES)]   # first tile pair A needs
TB = [((31 - 2 * c) * P) // 128 for c in range(NCORES)]  # first tile pair B needs
FIXED_A_T0 = 23   # fixed-A section: tiles 23..48 (>= max(TA)=22)
FIXED_B_T0 = 47   # fixed-B section: tiles 47,48 (>= max(TB)=47)
N_FIXED_A = NT - FIXED_A_T0          # 26
N_FLEX = 22
for _c in range(NCORES):
    assert (FIXED_A_T0 - TA[_c]) + (FIXED_B_T0 - TB[_c]) == N_FLEX
NN_SLOTS = N_FLEX + N_FIXED_A        # 48 packed stationary nn tiles
# fixed-A units that run before flex (their slots lead the packed layout so
# compute can start as soon as mov pair A + the first slots arrive).
N_FA_HEAD = int(os.environ.get("CL_HEAD", "1"))

_CACHE = {}


def _pairs(c):
    return (2 * c, 2 * c + 1), (30 - 2 * c, 31 - 2 * c)


def _flex_tiles(c):
    """Per-core flex q-tiles: pair-A leftovers then pair-B leftovers."""
    return list(range(TA[c], FIXED_A_T0)) + list(range(TB[c], FIXED_B_T0))


def _slot2tile(c):
    """Packed nn stationary layout: head fixed-A tiles, then per-core flex
    tiles, then the remaining fixed-A tiles (47,48 double as fixed-B)."""
    head = list(range(FIXED_A_T0, FIXED_A_T0 + 2 * N_FA_HEAD))
    rest = list(range(FIXED_A_T0 + 2 * N_FA_HEAD, NT))
    return head + _flex_tiles(c) + rest


def _schedule():
    """Core-independent unit list. Each unit: side 0 nn / 1 nd; banks: list
    of (slot, half); half 0 = moving cols 0:392 (pair A), 1 = 392:784 (pair
    B). Slot indexes 128-col groups of that side's packed stationary."""
    units = []
    if TRI:
        # fixed-A head: starts as soon as mov pair A + first slots land
        for u in range(N_FA_HEAD):
            units.append({"side": 0, "banks": [(2 * u, 0), (2 * u + 1, 0)]})
        # flex: one slot per unit (lets the DMA stream get ahead)
        for k in range(N_FLEX):
            s = 2 * N_FA_HEAD + k
            units.append({"side": 0, "banks": [(s, 0), (s, 1)]})
        # fixed-A rest
        for u in range((N_FIXED_A - 2 * N_FA_HEAD) // 2):
            s = 2 * N_FA_HEAD + N_FLEX + 2 * u
            units.append({"side": 0, "banks": [(s, 0), (s + 1, 0)]})
        # fixed-B: tiles 47,48 = last two slots, pair B
        units.append({"side": 0, "banks": [(NN_SLOTS - 2, 1), (NN_SLOTS - 1, 1)]})
    else:
        for t in range(NT):
            units.append({"side": 0, "banks": [(t, 0), (t, 1)]})
    for t in range(NT):
        units.append({"side": 1, "banks": [(t, 0), (t, 1)]})
    return units


SCHED = _schedule()
NN_UNITS = sum(1 for u in SCHED if u["side"] == 0)
NN_COLS = 4 * NN_UNITS
NCOLS = 4 * len(SCHED)
NN_STAT_COLS = (NN_SLOTS if TRI else NT) * 128
ND_STAT_COLS = NT * 128
# final-sum segments: (first_unit, last_unit_exclusive, is_pos). The last
# segment is small so only its drain sits in the serial tail.
ND0 = NN_UNITS
_NDTAIL = max(ND0, len(SCHED) - 6)
SEGMENTS = [
    (0, NN_UNITS, True),
    (ND0, _NDTAIL, False),
    (_NDTAIL, len(SCHED), False),
]


def _red_mode_seq(n):
    seq = [RED_PATTERN[u % len(RED_PATTERN)] for u in range(n)]
    # Last units drain on three different engines in parallel so the tail
    # isn't a single-engine backlog.
    seq[-3:] = ["C", "B", "V"]
    return seq


def _build_nc():
    import concourse.bacc as bacc
    import concourse.mybir as mybir
    import concourse.tile as tile

    f32 = mybir.dt.float32
    bf16 = mybir.dt.bfloat16
    u8 = mybir.dt.uint8
    f8 = mybir.dt.float8e4
    DR = mybir.MatmulPerfMode.DoubleRow
    HP = P // 2  # 98: max-halving split

    # Bacc (not plain Bass): its compile() runs move_matmul_waits_to_ldweights
    # + generate_event_semaphores, which legalize multi-semaphore waits for
    # the 1-wait-per-instruction ISA constraint.
    nc = bacc.Bacc("TRN2", target_bir_lowering=False, debug=False)

    mov_d = nc.dram_tensor("mov", [D, 4 * P], u8, kind="ExternalInput")
    stat_nn_d = nc.dram_tensor("stat_nn", [D, NN_STAT_COLS], u8, kind="ExternalInput")
    stat_nd_d = nc.dram_tensor("stat_nd", [D, ND_STAT_COLS], u8, kind="ExternalInput")
    wmask_d = nc.dram_tensor("wmask", [128, NN_COLS], f32, kind="ExternalInput")
    out_d = nc.dram_tensor("out", [1, len(SEGMENTS)], f32, kind="ExternalOutput")

    red_mode = _red_mode_seq(len(SCHED))

    with tile.TileContext(nc) as tc:
        with (
            tc.tile_pool(name="const", bufs=1) as const_pool,
            tc.tile_pool(name="statp", bufs=1) as stat_pool,
            tc.tile_pool(name="slots", bufs=1) as slot_pool,
            tc.tile_pool(name="stageB", bufs=3) as stageB_pool,
            tc.tile_pool(name="stageC", bufs=3) as stageC_pool,
            tc.tile_pool(name="psum", bufs=PSUM_BUFS, space="PSUM") as psum_pool,
        ):
            # Moving pair A first (the head units need only it), then the
            # first stationary slots, then pair B, then the rest.
            mov_sb = const_pool.tile([128, 6, 4 * P], u8)
            stat_nn_sb = stat_pool.tile([128, 6, NN_STAT_COLS], u8)
            stat_nd_sb = stat_pool.tile([128, 6, ND_STAT_COLS], u8)

            def _mov_chunk(lo, hi):
                nc.sync.dma_start(
                    mov_sb[:, :, lo:hi],
                    mov_d[:, lo:hi].rearrange("(c k) p -> k c p", k=128),
                )

            def _stat_chunks(dram, sbuf, bounds):
                for lo, hi in zip(bounds[:-1], bounds[1:]):
                    src = dram[:, 128 * lo : 128 * hi].rearrange(
                        "(c k) q -> k c q", k=128
                    )
                    nc.sync.dma_start(sbuf[:, :, 128 * lo : 128 * hi], src)

            nn_slots = NN_STAT_COLS // 128
            if N_FA_HEAD:
                h2 = 2 * N_FA_HEAD
                _mov_chunk(0, 392)
                _stat_chunks(stat_nn_d, stat_nn_sb, [0, h2])
                _mov_chunk(392, 784)
                _stat_chunks(stat_nn_d, stat_nn_sb,
                             [h2, h2 + 3, h2 + 7, h2 + 12, h2 + 18, 27, 35,
                              nn_slots])
            else:
                _mov_chunk(0, 784)
                _stat_chunks(stat_nn_d, stat_nn_sb,
                             [0, 3, 8, 14, 22, 30, 39, nn_slots])
            _stat_chunks(stat_nd_d, stat_nd_sb, [0, 12, 25, 37, NT])
            stat_sbs = (stat_nn_sb, stat_nd_sb)

            wmask_sb = const_pool.tile([128, NN_COLS], f32)
            nc.sync.dma_start(wmask_sb[:], wmask_d[:, :])

            ones_sb = const_pool.tile([128, 1], f32)
            nc.vector.memset(ones_sb[:], 1.0)
            mslots = slot_pool.tile([128, NCOLS], f32, name="mslots")
            acc = const_pool.tile([128, len(SEGMENTS)], f32)

            # PE p-state warmup on zeros while the first DMAs land.
            if WARM:
                zeros_sb = const_pool.tile([128, 2, 392], u8)
                nc.gpsimd.memset(zeros_sb[:], 0)
                pw = psum_pool.tile([128, 2, 512], f32, tag="ps")
                for _ in range(WARM):
                    nc.tensor.matmul(
                        pw[:, 0, 0:392],
                        zeros_sb[:, :, 0:128].bitcast(f8),
                        zeros_sb[:].bitcast(f8),
                        start=True,
                        stop=True,
                        perf_mode=DR,
                    )

            seg_of_unit = {}
            for si, (u0, u1, _) in enumerate(SEGMENTS):
                for u in range(u0, u1):
                    seg_of_unit[u] = si
            max_seg = max(4 * (u1 - u0) for u0, u1, _ in SEGMENTS)
            junk = slot_pool.tile([128, max_seg], f32, name="junk")

            for u, unit in enumerate(SCHED):
                ps = psum_pool.tile([128, 2, 512], f32, tag="ps")
                for b, (slot, half) in enumerate(unit["banks"]):
                    stat_sb = stat_sbs[unit["side"]]
                    for t3 in range(3):
                        lhsT = stat_sb[
                            :, 2 * t3 : 2 * t3 + 2, 128 * slot : 128 * (slot + 1)
                        ].bitcast(f8)
                        rhs = mov_sb[
                            :, 2 * t3 : 2 * t3 + 2, 392 * half : 392 * half + 392
                        ].bitcast(f8)
                        nc.tensor.matmul(
                            ps[:, b, 0:392],
                            lhsT,
                            rhs,
                            start=(t3 == 0),
                            stop=(t3 == 2),
                            perf_mode=DR,
                        )
                mview = ps[:, :, 0:392].rearrange("k b (i p) -> k b i p", p=P)
                mout = mslots[:, 4 * u : 4 * u + 4]
                mode = red_mode[u]
                if mode == "V":
                    nc.vector.reduce_max(
                        out=mout, in_=mview, axis=mybir.AxisListType.X
                    )
                elif mode == "B":
                    h = stageB_pool.tile([128, 2, 2, HP], f32, tag="hB")
                    nc.gpsimd.tensor_max(
                        h[:], mview[:, :, :, 0:HP], mview[:, :, :, HP:P]
                    )
                    nc.vector.reduce_max(
                        out=mout, in_=h[:], axis=mybir.AxisListType.X
                    )
                else:  # "C" / "D"
                    hc = stageC_pool.tile([128, 2, 2, P], bf16, tag="hC")
                    nc.scalar.copy(hc[:], mview)
                    h2 = stageC_pool.tile([128, 2, 2, HP], bf16, tag="hC2")
                    eng = nc.vector if mode == "C" else nc.gpsimd
                    eng.tensor_max(
                        h2[:], hc[:, :, :, 0:HP], hc[:, :, :, HP:P]
                    )
                    h3 = stageC_pool.tile([128, 2, 2, HP // 2], bf16, tag="hC3")
                    nc.vector.tensor_max(
                        h3[:], h2[:, :, :, 0 : HP // 2], h2[:, :, :, HP // 2 : HP]
                    )
                    nc.vector.reduce_max(
                        out=mout, in_=h3[:], axis=mybir.AxisListType.X
                    )

                # segment finals, fused accumulate, off the tail
                for si, (u0, u1, is_pos) in enumerate(SEGMENTS):
                    if u != u1 - 1:
                        continue
                    c0, c1 = 4 * u0, 4 * u1
                    if is_pos:
                        nc.vector.scalar_tensor_tensor(
                            out=junk[:, 0 : c1 - c0],
                            in0=mslots[:, c0:c1],
                            scalar=1.0,
                            in1=wmask_sb[:, c0:c1],
                            op0=mybir.AluOpType.mult,
                            op1=mybir.AluOpType.mult,
                            accum_out=acc[:, si : si + 1],
                        )
                    else:
                        nc.vector.tensor_scalar(
                            out=junk[:, 0 : c1 - c0],
                            in0=mslots[:, c0:c1],
                            scalar1=-MARGIN * SCALE * SCALE,
                            scalar2=0.0,
                            op0=mybir.AluOpType.add,
                            op1=mybir.AluOpType.max,
                            accum_out=acc[:, si : si + 1],
                        )

            # partition reduction via ones-matmul into a rotated psum tile
            ps_f = psum_pool.tile([128, 2, 512], f32, tag="ps")
            nc.tensor.matmul(
                ps_f[0:1, 0, 0 : len(SEGMENTS)],
                ones_sb[:],
                acc[:],
                start=True,
                stop=True,
            )
            out_sb = const_pool.tile([1, len(SEGMENTS)], f32)
            nc.vector.tensor_copy(out_sb[:], ps_f[0:1, 0, 0 : len(SEGMENTS)])
            nc.sync.dma_start(out_d[:, :], out_sb[:])

    nc.compile()
    return nc


def _quant(n):
    """[*, D] fp32 normalized -> fp8e4m3 bytes of n*SCALE."""
    import ml_dtypes

    return (n * SCALE).astype(ml_dtypes.float8_e4m3).view(np.uint8)


def _statq(n):
    """[32,P,D] -> d-major [D, 32*P] (j-major q axis)."""
    return np.ascontiguousarray(n.transpose(2, 0, 1).reshape(D, N_IMG * P))


def _build_in_maps(normal_embed, defect_embed):
    x1 = np.asarray(normal_embed, dtype=np.float32)
    x2 = np.asarray(defect_embed, dtype=np.float32)
    n1 = x1 / (np.sqrt(np.sum(x1 * x1, axis=-1, keepdims=True)) + EPS)
    n2 = x2 / (np.sqrt(np.sum(x2 * x2, axis=-1, keepdims=True)) + EPS)

    q1 = _statq(_quant(n1))  # [D, 6272] uint8 view of fp8
    q2 = _statq(_quant(n2))

    jq = np.arange(NT * 128) // P  # j image per stationary q

    in_maps = []
    for c in range(NCORES):
        pa, pb = _pairs(c)
        imgs = [pa[0], pa[1], pb[0], pb[1]]
        mov = np.ascontiguousarray(
            np.concatenate([q1[:, i * P : (i + 1) * P] for i in imgs], axis=1)
        )

        if TRI:
            slot2tile = _slot2tile(c)
            stat_nn = np.ascontiguousarray(
                np.concatenate(
                    [q1[:, 128 * t : 128 * (t + 1)] for t in slot2tile], axis=1
                )
            )
        else:
            stat_nn = q1
            slot2tile = list(range(NT))

        wm = np.zeros((128, NN_COLS), dtype=np.float32)
        for u, unit in enumerate(SCHED):
            if unit["side"] != 0:
                continue
            for b, (slot, half) in enumerate(unit["banks"]):
                t = slot2tile[slot]
                pair = pa if half == 0 else pb
                # flex double-count guard: the fixed sections are the unique
                # cover for tiles >= FIXED_A_T0 (pair A) / >= FIXED_B_T0
                # (pair B), so flex contributes pair A only below FIXED_A_T0
                # and pair B only at/above it (flex-B tiles are < FIXED_B_T0).
                if TRI and N_FA_HEAD <= u < N_FA_HEAD + N_FLEX:
                    ok = (t < FIXED_A_T0) if half == 0 else (t >= FIXED_A_T0)
                    if not ok:
                        continue
                q = 128 * t + np.arange(128)
                for m in range(2):
                    col = 4 * u + 2 * b + m
                    wm[:, col] = (jq[q] > pair[m]).astype(np.float32)
        assert int(wm.sum()) == 62 * P, (c, int(wm.sum()))

        in_maps.append(
            {
                "mov": mov,
                "stat_nn": stat_nn,
                "stat_nd": q2,
                "wmask": np.ascontiguousarray(wm),
            }
        )
    return in_maps


def _get_nc():
    key = ("nc", TRI, RED_PATTERN, WARM, PSUM_BUFS)
    if key not in _CACHE:
        _CACHE[key] = _build_nc()
    return _CACHE[key]


def _run_on_device(in_maps, trace=False):
    from concourse.bass_utils import run_bass_kernel_spmd

    nc = _get_nc()
    return run_bass_kernel_spmd(
        nc, in_maps, core_ids=list(range(NCORES)), trace=trace
    )


def _combine(results):
    s_pos = 0.0
    s_neg = 0.0
    for r in results:
        o = np.asarray(r["out"], dtype=np.float64).reshape(-1)
        for si, (_, _, is_pos) in enumerate(SEGMENTS):
            if is_pos:
                s_pos += float(o[si])
            else:
                s_neg += float(o[si])
    s2 = SCALE * SCALE
    loss = 1.0 - s_pos / s2 / (NPAIRS * P) + s_neg / s2 / (N_IMG * N_IMG * P)
    return np.float32(loss)


def kernel(normal_embed, defect_embed):
    in_maps = _build_in_maps(normal_embed, defect_embed)
    res = _run_on_device(in_maps, trace=False)
    return _combine(res.results)
